# revision 43
# baseline (speedup 1.0000x reference)
"""Attentional Factorization Machine kernel for 8 Trainium2 NeuronCores.

Data-parallel over batch: 1024 rows -> 128 per core. The 780 field-pair
products per row are processed in TWO PASSES over pair columns (deltas 1-10
= pairs 0:400, deltas 11-20 = pairs 400:780). Per pass: hp products are
built by cyclic-delta enumeration (fp16, DVE 2x, merged strided instruction
per parity with a stride-0 broadcast first operand); the attention MLP mm1
runs on the PE with W stationary (one <=400-column matmul per row, fp32 into
one PSUM bank of a 3-row mega-tile); relu+bias eviction runs on 3-row
batches (PSUM->SBUF) split between the scalar engine and DVE; per-pair
scores and p_w projections accumulate in fp32 via one-hot stationary matmuls
packed across PE column groups (rows processed in quads, one row per column
group). The pass-0 accumulators are copied to SBUF between passes so each
pass needs only 2 persistent PSUM banks, leaving 6 banks for aw tiles.
Softmax + combine happen on-chip in a [128, 780] layout; exp is applied
without max subtraction (logits are bounded, softmax is shift-invariant).
"""
import sys
for _p in ("/opt/trn_rl_repo",):
    if _p not in sys.path:
        sys.path.insert(0, _p)

import numpy as np

import concourse.bass as bass
import concourse.bacc as bacc
import concourse.mybir as mybir
import concourse.tile as tile
from concourse.ap import AP

F32 = mybir.dt.float32
F16 = mybir.dt.float16
AF = mybir.ActivationFunctionType
ALU = mybir.AluOpType
AXIS = mybir.AxisListType

FLD = 40
NDELTA = 20
P = 780
H0 = 400   # pass-0 pair columns (deltas 1..10)
H1 = 380   # pass-1 real pair columns (deltas 11..20, d=20 has 20 junk)
ARQ = 400  # hp arena per-row stride (one pass half, incl. junk pad)

_DMA_CHUNKS = (8, 8, 16, 16, 16, 16, 16, 16, 16)  # row chunks, 8-aligned


def _win(base, dims):
    """Raw AP from base slice with explicit [stride, size] free dims."""
    pdim = list(base.ap[0])
    return AP(base.tensor, base.offset, [pdim] + [list(d) for d in dims])


def build(nc, B_c=128, dve_mod=5):
    assert B_c == 128

    xTa_d = nc.dram_tensor("xTa", [128, B_c, 60], F16, kind="ExternalInput").ap()
    xTb_d = nc.dram_tensor("xTb", [128, B_c, 60], F16, kind="ExternalInput").ap()
    wT_d = nc.dram_tensor("wT", [128, 128], F16, kind="ExternalInput").ap()
    bias_d = nc.dram_tensor("bias", [128, 1], F32, kind="ExternalInput").ap()
    Zh_d = nc.dram_tensor("Zh", [128, 64], F16, kind="ExternalInput").ap()
    Zg_d = nc.dram_tensor("Zg", [128, 64], F16, kind="ExternalInput").ap()
    pb_d = nc.dram_tensor("pb", [128, 1], F32, kind="ExternalInput").ap()
    out_d = nc.dram_tensor("out", [B_c, 1], F32, kind="ExternalOutput").ap()

    with tile.TileContext(nc) as tc:
        with (
            tc.tile_pool(name="const", bufs=1) as cpool,
            tc.tile_pool(name="relu", bufs=12) as rpool,
            tc.tile_pool(name="awps", bufs=2, space="PSUM") as awpool,
            tc.tile_pool(name="accps", bufs=1, space="PSUM") as accpool,
        ):
            wT_s = cpool.tile([128, 128], F16, tag="wT")
            bias_s = cpool.tile([128, 1], F32, tag="bias")
            Zh_s = cpool.tile([128, 64], F16, tag="Zh")
            Zg_s = cpool.tile([128, 64], F16, tag="Zg")
            pb_s = cpool.tile([128, 1], F32, tag="pb")
            xTa = cpool.tile([128, B_c, 60], F16, tag="xTa")
            xTb = cpool.tile([128, B_c, 60], F16, tag="xTb")
            arena = cpool.tile([128, B_c, ARQ], F16, tag="hparena")
            sc0_s = cpool.tile([128, H0], F32, tag="sc0_s")
            g0_s = cpool.tile([128, H0], F32, tag="g0_s")
            warm0 = cpool.tile([128, 1], F32, tag="warm0")
            warm1 = cpool.tile([128, 1], F32, tag="warm1")
            warmw = cpool.tile([128, 64], F16, tag="warmw")

            # Warmups with no data deps: load the exp table set on ACT
            # (contains relu too, so exactly one ACT_TABLE_LOAD, at t~0) and
            # keep the PE busy through the startup DMA window so the HAM
            # clock gate reaches 2.4 GHz before the first real matmul.
            nc.vector.memset(warm0[:], 0.0)
            nc.scalar.activation(warm1[:], warm0[:], AF.Exp)
            nc.vector.memset(warmw[:], 0.0)
            wmt = awpool.tile([128, 1536], F32, tag="aw")
            for wi in range(100):
                bi = nc.tensor.matmul(wmt[0:64, 0:64], warmw[:], warmw[:],
                                      start=True, stop=True)
                if wi > 0:
                    bi.ins.ldweights = False

            # first two x chunks, then the consts needed earliest; remaining
            # chunks are issued one chunk ahead of consumption
            starts = np.cumsum((0,) + _DMA_CHUNKS[:-1]).tolist()
            chunk_of = dict(zip(starts, _DMA_CHUNKS))
            for s in starts[:2]:
                ch = chunk_of[s]
                nc.sync.dma_start(xTa[:, s:s + ch, :], xTa_d[:, s:s + ch, :])
                nc.sync.dma_start(xTb[:, s:s + ch, :], xTb_d[:, s:s + ch, :])
            nc.sync.dma_start(wT_s[:], wT_d[:])
            nc.sync.dma_start(bias_s[:], bias_d[:])
            nc.sync.dma_start(Zh_s[:], Zh_d[:])
            nc.sync.dma_start(Zg_s[:], Zg_d[:])
            nc.sync.dma_start(pb_s[:], pb_d[:])
            prefetch_of = {a: b for a, b in zip(starts[:-1], starts[2:])}

            def emit_hp(pass_, r0, nr, parity):
                """Merged multiply over this pass's deltas of one parity for
                rows r0:r0+nr into the arena (local col (d-dlo)*40).

                in0 = x[0:40] broadcast over deltas; in1 = shifted window.
                d=20 writes 40 cols, the last 20 junk -> arena 380:400 pad.
                """
                dlo = 1 + 10 * pass_
                deltas = [d for d in range(dlo, dlo + 10) if d % 2 == parity]
                nd = len(deltas)
                d0 = deltas[0]
                col0 = (d0 - dlo) * FLD
                ob = arena[:, r0:r0 + nr, col0:col0 + 1]
                out_ap = _win(ob, [[ARQ, nr], [2 * FLD, nd], [1, FLD]])
                i0b = xTa[:, r0:r0 + nr, 0:1]
                in0 = _win(i0b, [[60, nr], [0, nd], [1, FLD]])
                if d0 % 2 == 0:
                    i1b = xTa[:, r0:r0 + nr, d0:d0 + 1]
                else:
                    i1b = xTb[:, r0:r0 + nr, d0 - 1:d0]
                in1 = _win(i1b, [[60, nr], [2, nd], [1, FLD]])
                bi = nc.vector.tensor_mul(out_ap, in0, in1)
                # defer hp builds behind ready evicts on the DVE: evictions
                # gate the PE's aw-tile reuse, hp only gates rows ~8 ahead
                bi.ins.bass_priority = tc.cur_priority + 150

            def emit_evict(awt, relu3, n, W, on_dve):
                aw_v = awt[:].rearrange("a (u q) -> a u q", q=512)[:, 0:n, 0:W]
                rl_v = relu3[:].rearrange("a (u q) -> a u q", q=ARQ)[:, 0:n, 0:W]
                if on_dve:
                    nc.vector.tensor_scalar(
                        rl_v, aw_v, bias_s[:], 0.0, op0=ALU.add, op1=ALU.max)
                else:
                    nc.scalar.activation(rl_v, aw_v, AF.Relu, bias=bias_s[:])

            def emit_scg(rec, sc_t, g_t, W):
                t, rows = rec
                st, sp = (t == 0), (t == 31)
                for purpose in (0, 1):
                    for (k, relu_mv) in rows:
                        j = k % 4
                        if purpose == 0:
                            dst, Z, mov = sc_t, Zh_s, relu_mv
                        else:
                            dst, Z, mov = g_t, Zg_s, arena[:, k, 0:W]
                        nc.tensor.matmul(
                            dst[32 * j:32 * j + 32, 0:W],
                            Z[:, 32 - t:64 - t],
                            mov,
                            start=st, stop=sp,
                            tile_position=(0, 32 * j),
                            skip_group_check=True,
                        )

            class PassState:
                def __init__(self, pass_):
                    self.pass_ = pass_
                    self.W = H0 if pass_ == 0 else H1
                    self.sc_t = accpool.tile([128, 512], F32, tag="sc")
                    self.g_t = accpool.tile([128, 512], F32, tag="g")
                    self.scg_q = []
                    self.quad_rows = []
                    self.awt = self.relu3 = None
                    self.grp = 0

            def do_row(st, k):
                W = st.W
                slot = k % 3
                if slot == 0:
                    st.awt = awpool.tile([128, 1536], F32, tag="aw")
                    st.relu3 = rpool.tile([128, 3 * ARQ], F16, tag="relu3")
                bi = nc.tensor.matmul(
                    st.awt[:, 512 * slot:512 * slot + W],
                    wT_s[:],
                    arena[:, k, 0:W],
                    start=True, stop=True,
                )
                if k % 4 != 0:
                    bi.ins.ldweights = False
                if slot == 2 or k == B_c - 1:
                    on_dve = (st.grp % dve_mod == 2)
                    emit_evict(st.awt, st.relu3, slot + 1, W, on_dve)
                    st.grp += 1
                st.quad_rows.append(
                    (k, st.relu3[:].rearrange("a (u q) -> a u q", q=ARQ)
                     [:, slot, 0:W]))
                if len(st.quad_rows) == 4:
                    t = st.quad_rows[0][0] // 4
                    if len(st.scg_q) >= 6:
                        emit_scg(st.scg_q.pop(0), st.sc_t, st.g_t, W)
                    st.scg_q.append((t, st.quad_rows))
                    st.quad_rows = []

            st0 = PassState(0)
            st1 = PassState(1)
            sc_t1, g_t1 = st1.sc_t, st1.g_t

            hp_starts0 = [0, 4] + list(range(8, 128, 8))
            hp_idx = 0
            for k in range(B_c):
                if k in prefetch_of:
                    s = prefetch_of[k]
                    ch = chunk_of[s]
                    nc.sync.dma_start(xTa[:, s:s + ch, :],
                                      xTa_d[:, s:s + ch, :])
                    nc.sync.dma_start(xTb[:, s:s + ch, :],
                                      xTb_d[:, s:s + ch, :])
                if k == 88:
                    # head of pass-1 hp: arena rows 0:24 are long done being
                    # read; build them while pass-0 finishes
                    for r0, r1 in ((0, 8), (8, 16), (16, 24)):
                        emit_hp(1, r0, r1 - r0, 1)
                        emit_hp(1, r0, r1 - r0, 0)
                while hp_idx < len(hp_starts0) and hp_starts0[hp_idx] <= k + 16:
                    r0 = hp_starts0[hp_idx]
                    r1 = hp_starts0[hp_idx + 1] if hp_idx + 1 < len(
                        hp_starts0) else 128
                    emit_hp(0, r0, r1 - r0, 1)
                    emit_hp(0, r0, r1 - r0, 0)
                    hp_idx += 1
                do_row(st0, k)

            # pass-1 head: its hp is prebuilt and its scg queue just fills
            # (no emissions), giving the PE ready work while pass-0's tail
            # evicts and queued scg quads drain
            for k in range(24):
                do_row(st1, k)

            while st0.scg_q:
                emit_scg(st0.scg_q.pop(0), st0.sc_t, st0.g_t, st0.W)
            # free the accumulator banks for pass 1
            nc.scalar.copy(sc0_s[:], st0.sc_t[:, 0:H0])
            nc.vector.tensor_copy(g0_s[:], st0.g_t[:, 0:H0])

            # pass-0 half of the softmax tail: ready now, runs during pass 1
            exp_s = cpool.tile([128, P], F32, tag="exp_s")
            junk = cpool.tile([128, P], F32, tag="junk")
            den2 = cpool.tile([128, 2], F32, tag="den2")
            num2 = cpool.tile([128, 2], F32, tag="num2")
            nc.scalar.activation(exp_s[:, 0:H0], sc0_s[:], AF.Exp,
                                 accum_out=den2[:, 0:1])
            nc.vector.scalar_tensor_tensor(
                junk[:, 0:H0], exp_s[:, 0:H0], 1.0, g0_s[:],
                op0=ALU.mult, op1=ALU.mult, accum_out=num2[:, 0:1])

            hp_starts1 = list(range(24, 128, 8))  # 0:24 pre-built
            hp_idx = 0
            for k in range(24, B_c):
                while hp_idx < len(hp_starts1) and hp_starts1[hp_idx] <= k + 16:
                    r0 = hp_starts1[hp_idx]
                    r1 = hp_starts1[hp_idx + 1] if hp_idx + 1 < len(
                        hp_starts1) else 128
                    emit_hp(1, r0, r1 - r0, 1)
                    emit_hp(1, r0, r1 - r0, 0)
                    hp_idx += 1
                do_row(st1, k)

            while st1.scg_q:
                emit_scg(st1.scg_q.pop(0), st1.sc_t, st1.g_t, st1.W)

            # ---- softmax tail ----
            # logits are bounded (|sc| <~ 45) so exp without max subtraction
            # is safe in fp32 and softmax is exactly shift-invariant.
            denom = cpool.tile([128, 1], F32, tag="denom")
            rden = cpool.tile([128, 1], F32, tag="rden")
            numer = cpool.tile([128, 1], F32, tag="numer")
            outc = cpool.tile([128, 1], F32, tag="outc")

            nc.scalar.activation(exp_s[:, H0:P], sc_t1[:, 0:H1], AF.Exp,
                                 accum_out=den2[:, 1:2])
            nc.vector.scalar_tensor_tensor(
                junk[:, H0:P], exp_s[:, H0:P], 1.0, g_t1[:, 0:H1],
                op0=ALU.mult, op1=ALU.mult, accum_out=num2[:, 1:2])
            nc.vector.tensor_reduce(numer[:], num2[:], axis=AXIS.X, op=ALU.add)
            nc.vector.tensor_reduce(denom[:], den2[:], axis=AXIS.X, op=ALU.add)
            nc.vector.reciprocal(rden[:], denom[:])
            nc.vector.tensor_mul(outc[:], numer[:], rden[:])
            nc.vector.tensor_scalar_add(outc[:], outc[:], pb_s[:])
            nc.sync.dma_start(out_d[:], outc[:])

    nc.compile()
    return nc


def make_nc(B_c=128, dve_mod=5):
    nc = bacc.Bacc("TRN2", target_bir_lowering=False, debug=False)
    build(nc, B_c=B_c, dve_mod=dve_mod)
    return nc


def perm_for(B_c=128, blocks=None):
    """perm[slot] = global b stored at SBUF slot.

    Slot k belongs to quad k//4 (the one-hot position) and column group
    k%4, so it accumulates into output partition 32*(k%4) + k//4.
    """
    k = np.arange(B_c)
    return 32 * (k % 4) + k // 4


def host_prep_consts(attn_w_w, attn_w_b, attn_h_w, attn_h_b, attn_p_w, attn_p_b):
    wT = np.ascontiguousarray(attn_w_w.T).astype(np.float16)
    bias = attn_w_b.reshape(128, 1).astype(np.float32)
    Zh = np.zeros((128, 64), np.float16)
    Zh[:, 32] = attn_h_w[0].astype(np.float16)
    Zg = np.zeros((128, 64), np.float16)
    Zg[:, 32] = attn_p_w[0].astype(np.float16)
    pb = np.full((128, 1), np.float32(attn_p_b[0]), np.float32)
    return {"wT": wT, "bias": bias, "Zh": Zh, "Zg": Zg, "pb": pb}


def host_prep_x(x_slice, blocks=None):
    # [B_c, F, E] -> two pre-shifted fp16 copies [E, B_c(perm), 60]
    xT = x_slice.transpose(2, 0, 1).astype(np.float16)
    xT = xT[:, perm_for(x_slice.shape[0]), :]
    B_c = x_slice.shape[0]
    xa = np.zeros((128, B_c, 60), np.float16)
    xa[:, :, 0:40] = xT
    xa[:, :, 40:60] = xT[:, :, 0:20]
    xb = np.zeros((128, B_c, 60), np.float16)
    xb[:, :, 0:59] = xa[:, :, 1:60]
    return np.ascontiguousarray(xa), np.ascontiguousarray(xb)


_NC_CACHE = {}


def _get_nc():
    if "nc" not in _NC_CACHE:
        _NC_CACHE["nc"] = make_nc(B_c=128)
    return _NC_CACHE["nc"]


def kernel(x, attn_w_w, attn_w_b, attn_h_w, attn_h_b, attn_p_w, attn_p_b,
           _trace=False):
    from concourse.bass_utils import run_bass_kernel_spmd
    x = np.asarray(x, np.float32)
    consts = host_prep_consts(np.asarray(attn_w_w), np.asarray(attn_w_b),
                              np.asarray(attn_h_w), np.asarray(attn_h_b),
                              np.asarray(attn_p_w), np.asarray(attn_p_b))
    in_maps = []
    for c in range(8):
        m = dict(consts)
        m["xTa"], m["xTb"] = host_prep_x(x[128 * c:128 * (c + 1)])
        in_maps.append(m)
    nc = _get_nc()
    res = run_bass_kernel_spmd(nc, in_maps, list(range(8)), trace=_trace)
    out = np.concatenate([res.results[c]["out"][:, 0] for c in range(8)])
    if _trace:
        return out.astype(np.float32), res
    return out.astype(np.float32)


# revision 44
# speedup vs baseline: 1.0190x; 1.0190x over previous
"""Attentional Factorization Machine kernel for 8 Trainium2 NeuronCores.

Data-parallel over batch: 1024 rows -> 128 per core. The 780 field-pair
products per row are processed in TWO PASSES over pair columns (deltas 1-10
= pairs 0:400, deltas 11-20 = pairs 400:780). Per pass: hp products are
built by cyclic-delta enumeration (fp16, DVE 2x, merged strided instruction
per parity with a stride-0 broadcast first operand); the attention MLP mm1
runs on the PE with W stationary (one <=400-column matmul per row, fp32 into
one PSUM bank of a 3-row mega-tile); relu+bias eviction runs on 3-row
batches (PSUM->SBUF) split between the scalar engine and DVE; per-pair
scores and p_w projections accumulate in fp32 via one-hot stationary matmuls
packed across PE column groups (rows processed in quads, one row per column
group). The pass-0 accumulators are copied to SBUF between passes so each
pass needs only 2 persistent PSUM banks, leaving 6 banks for aw tiles.
Softmax + combine happen on-chip in a [128, 780] layout; exp is applied
without max subtraction (logits are bounded, softmax is shift-invariant).
"""
import sys
for _p in ("/opt/trn_rl_repo",):
    if _p not in sys.path:
        sys.path.insert(0, _p)

import numpy as np

import concourse.bass as bass
import concourse.bacc as bacc
import concourse.mybir as mybir
import concourse.tile as tile
from concourse.ap import AP

F32 = mybir.dt.float32
F16 = mybir.dt.float16
AF = mybir.ActivationFunctionType
ALU = mybir.AluOpType
AXIS = mybir.AxisListType

FLD = 40
NDELTA = 20
P = 780
H0 = 400   # pass-0 pair columns (deltas 1..10)
H1 = 380   # pass-1 real pair columns (deltas 11..20, d=20 has 20 junk)
ARQ = 400  # hp arena per-row stride (one pass half, incl. junk pad)

_DMA_CHUNKS = (8, 8, 16, 16, 16, 16, 16, 16, 16)  # row chunks, 8-aligned


def _win(base, dims):
    """Raw AP from base slice with explicit [stride, size] free dims."""
    pdim = list(base.ap[0])
    return AP(base.tensor, base.offset, [pdim] + [list(d) for d in dims])


def build(nc, B_c=128, dve_mod=6):
    assert B_c == 128

    xTa_d = nc.dram_tensor("xTa", [128, B_c, 60], F16, kind="ExternalInput").ap()
    xTb_d = nc.dram_tensor("xTb", [128, B_c, 60], F16, kind="ExternalInput").ap()
    wT_d = nc.dram_tensor("wT", [128, 128], F16, kind="ExternalInput").ap()
    bias_d = nc.dram_tensor("bias", [128, 1], F32, kind="ExternalInput").ap()
    Zh_d = nc.dram_tensor("Zh", [128, 64], F16, kind="ExternalInput").ap()
    Zg_d = nc.dram_tensor("Zg", [128, 64], F16, kind="ExternalInput").ap()
    pb_d = nc.dram_tensor("pb", [128, 1], F32, kind="ExternalInput").ap()
    out_d = nc.dram_tensor("out", [B_c, 1], F32, kind="ExternalOutput").ap()

    with tile.TileContext(nc) as tc:
        with (
            tc.tile_pool(name="const", bufs=1) as cpool,
            tc.tile_pool(name="relu", bufs=12) as rpool,
            tc.tile_pool(name="awps", bufs=2, space="PSUM") as awpool,
            tc.tile_pool(name="accps", bufs=1, space="PSUM") as accpool,
        ):
            wT_s = cpool.tile([128, 128], F16, tag="wT")
            bias_s = cpool.tile([128, 1], F32, tag="bias")
            Zh_s = cpool.tile([128, 64], F16, tag="Zh")
            Zg_s = cpool.tile([128, 64], F16, tag="Zg")
            pb_s = cpool.tile([128, 1], F32, tag="pb")
            xTa = cpool.tile([128, B_c, 60], F16, tag="xTa")
            xTb = cpool.tile([128, B_c, 60], F16, tag="xTb")
            arena = cpool.tile([128, B_c, ARQ], F16, tag="hparena")
            sc0_s = cpool.tile([128, H0], F32, tag="sc0_s")
            g0_s = cpool.tile([128, H0], F32, tag="g0_s")
            warm0 = cpool.tile([128, 1], F32, tag="warm0")
            warm1 = cpool.tile([128, 1], F32, tag="warm1")
            warmw = cpool.tile([128, 64], F16, tag="warmw")

            # Warmups with no data deps: load the exp table set on ACT
            # (contains relu too, so exactly one ACT_TABLE_LOAD, at t~0) and
            # keep the PE busy through the startup DMA window so the HAM
            # clock gate reaches 2.4 GHz before the first real matmul.
            nc.vector.memset(warm0[:], 0.0)
            nc.scalar.activation(warm1[:], warm0[:], AF.Exp)
            nc.vector.memset(warmw[:], 0.0)
            wmt = awpool.tile([128, 1536], F32, tag="aw")
            for wi in range(100):
                bi = nc.tensor.matmul(wmt[0:64, 0:64], warmw[:], warmw[:],
                                      start=True, stop=True)
                if wi > 0:
                    bi.ins.ldweights = False

            # first two x chunks, then the consts needed earliest; remaining
            # chunks are issued one chunk ahead of consumption
            starts = np.cumsum((0,) + _DMA_CHUNKS[:-1]).tolist()
            chunk_of = dict(zip(starts, _DMA_CHUNKS))
            for s in starts[:2]:
                ch = chunk_of[s]
                nc.sync.dma_start(xTa[:, s:s + ch, :], xTa_d[:, s:s + ch, :])
                nc.sync.dma_start(xTb[:, s:s + ch, :], xTb_d[:, s:s + ch, :])
            nc.sync.dma_start(wT_s[:], wT_d[:])
            nc.sync.dma_start(bias_s[:], bias_d[:])
            nc.sync.dma_start(Zh_s[:], Zh_d[:])
            nc.sync.dma_start(Zg_s[:], Zg_d[:])
            nc.sync.dma_start(pb_s[:], pb_d[:])
            prefetch_of = {a: b for a, b in zip(starts[:-1], starts[2:])}

            def emit_hp(pass_, r0, nr, parity):
                """Merged multiply over this pass's deltas of one parity for
                rows r0:r0+nr into the arena (local col (d-dlo)*40).

                in0 = x[0:40] broadcast over deltas; in1 = shifted window.
                d=20 writes 40 cols, the last 20 junk -> arena 380:400 pad.
                """
                dlo = 1 + 10 * pass_
                deltas = [d for d in range(dlo, dlo + 10) if d % 2 == parity]
                nd = len(deltas)
                d0 = deltas[0]
                col0 = (d0 - dlo) * FLD
                ob = arena[:, r0:r0 + nr, col0:col0 + 1]
                out_ap = _win(ob, [[ARQ, nr], [2 * FLD, nd], [1, FLD]])
                i0b = xTa[:, r0:r0 + nr, 0:1]
                in0 = _win(i0b, [[60, nr], [0, nd], [1, FLD]])
                if d0 % 2 == 0:
                    i1b = xTa[:, r0:r0 + nr, d0:d0 + 1]
                else:
                    i1b = xTb[:, r0:r0 + nr, d0 - 1:d0]
                in1 = _win(i1b, [[60, nr], [2, nd], [1, FLD]])
                bi = nc.vector.tensor_mul(out_ap, in0, in1)
                # defer hp builds behind ready evicts on the DVE: evictions
                # gate the PE's aw-tile reuse, hp only gates rows ~8 ahead
                bi.ins.bass_priority = tc.cur_priority + 150

            def emit_evict(awt, relu3, n, W, on_dve):
                aw_v = awt[:].rearrange("a (u q) -> a u q", q=512)[:, 0:n, 0:W]
                rl_v = relu3[:].rearrange("a (u q) -> a u q", q=ARQ)[:, 0:n, 0:W]
                if on_dve:
                    nc.vector.tensor_scalar(
                        rl_v, aw_v, bias_s[:], 0.0, op0=ALU.add, op1=ALU.max)
                else:
                    nc.scalar.activation(rl_v, aw_v, AF.Relu, bias=bias_s[:])

            def emit_scg(rec, sc_t, g_t, W):
                t, rows = rec
                st, sp = (t == 0), (t == 31)
                for purpose in (0, 1):
                    for (k, relu_mv) in rows:
                        j = k % 4
                        if purpose == 0:
                            dst, Z, mov = sc_t, Zh_s, relu_mv
                        else:
                            dst, Z, mov = g_t, Zg_s, arena[:, k, 0:W]
                        nc.tensor.matmul(
                            dst[32 * j:32 * j + 32, 0:W],
                            Z[:, 32 - t:64 - t],
                            mov,
                            start=st, stop=sp,
                            tile_position=(0, 32 * j),
                            skip_group_check=True,
                        )

            class PassState:
                def __init__(self, pass_):
                    self.pass_ = pass_
                    self.W = H0 if pass_ == 0 else H1
                    self.sc_t = accpool.tile([128, 512], F32, tag="sc")
                    self.g_t = accpool.tile([128, 512], F32, tag="g")
                    self.scg_q = []
                    self.quad_rows = []
                    self.awt = self.relu3 = None
                    self.grp = 0

            def do_row(st, k):
                W = st.W
                slot = k % 3
                if slot == 0:
                    st.awt = awpool.tile([128, 1536], F32, tag="aw")
                    st.relu3 = rpool.tile([128, 3 * ARQ], F16, tag="relu3")
                bi = nc.tensor.matmul(
                    st.awt[:, 512 * slot:512 * slot + W],
                    wT_s[:],
                    arena[:, k, 0:W],
                    start=True, stop=True,
                )
                if k % 4 != 0:
                    bi.ins.ldweights = False
                if slot == 2 or k == B_c - 1:
                    on_dve = (st.grp % dve_mod == 2)
                    emit_evict(st.awt, st.relu3, slot + 1, W, on_dve)
                    st.grp += 1
                st.quad_rows.append(
                    (k, st.relu3[:].rearrange("a (u q) -> a u q", q=ARQ)
                     [:, slot, 0:W]))
                if len(st.quad_rows) == 4:
                    t = st.quad_rows[0][0] // 4
                    if len(st.scg_q) >= 6:
                        emit_scg(st.scg_q.pop(0), st.sc_t, st.g_t, W)
                    st.scg_q.append((t, st.quad_rows))
                    st.quad_rows = []

            st0 = PassState(0)
            st1 = PassState(1)
            sc_t1, g_t1 = st1.sc_t, st1.g_t

            hp_starts0 = [0, 4] + list(range(8, 128, 8))
            hp_idx = 0
            for k in range(B_c):
                if k in prefetch_of:
                    s = prefetch_of[k]
                    ch = chunk_of[s]
                    nc.sync.dma_start(xTa[:, s:s + ch, :],
                                      xTa_d[:, s:s + ch, :])
                    nc.sync.dma_start(xTb[:, s:s + ch, :],
                                      xTb_d[:, s:s + ch, :])
                if k == 88:
                    # head of pass-1 hp: arena rows 0:24 are long done being
                    # read; build them while pass-0 finishes
                    for r0, r1 in ((0, 8), (8, 16), (16, 24)):
                        emit_hp(1, r0, r1 - r0, 1)
                        emit_hp(1, r0, r1 - r0, 0)
                while hp_idx < len(hp_starts0) and hp_starts0[hp_idx] <= k + 16:
                    r0 = hp_starts0[hp_idx]
                    r1 = hp_starts0[hp_idx + 1] if hp_idx + 1 < len(
                        hp_starts0) else 128
                    emit_hp(0, r0, r1 - r0, 1)
                    emit_hp(0, r0, r1 - r0, 0)
                    hp_idx += 1
                do_row(st0, k)

            # pass-1 head: its hp is prebuilt and its scg queue just fills
            # (no emissions), giving the PE ready work while pass-0's tail
            # evicts and queued scg quads drain
            for k in range(24):
                do_row(st1, k)

            while st0.scg_q:
                emit_scg(st0.scg_q.pop(0), st0.sc_t, st0.g_t, st0.W)
            # free the accumulator banks for pass 1
            nc.scalar.copy(sc0_s[:], st0.sc_t[:, 0:H0])
            nc.vector.tensor_copy(g0_s[:], st0.g_t[:, 0:H0])

            # pass-0 half of the softmax tail: ready now, runs during pass 1
            exp_s = cpool.tile([128, P], F32, tag="exp_s")
            junk = cpool.tile([128, P], F32, tag="junk")
            den2 = cpool.tile([128, 2], F32, tag="den2")
            num2 = cpool.tile([128, 2], F32, tag="num2")
            nc.scalar.activation(exp_s[:, 0:H0], sc0_s[:], AF.Exp,
                                 accum_out=den2[:, 0:1])
            nc.vector.scalar_tensor_tensor(
                junk[:, 0:H0], exp_s[:, 0:H0], 1.0, g0_s[:],
                op0=ALU.mult, op1=ALU.mult, accum_out=num2[:, 0:1])

            hp_starts1 = list(range(24, 128, 8))  # 0:24 pre-built
            hp_idx = 0
            for k in range(24, B_c):
                while hp_idx < len(hp_starts1) and hp_starts1[hp_idx] <= k + 16:
                    r0 = hp_starts1[hp_idx]
                    r1 = hp_starts1[hp_idx + 1] if hp_idx + 1 < len(
                        hp_starts1) else 128
                    emit_hp(1, r0, r1 - r0, 1)
                    emit_hp(1, r0, r1 - r0, 0)
                    hp_idx += 1
                do_row(st1, k)

            while st1.scg_q:
                emit_scg(st1.scg_q.pop(0), st1.sc_t, st1.g_t, st1.W)

            # ---- softmax tail ----
            # logits are bounded (|sc| <~ 45) so exp without max subtraction
            # is safe in fp32 and softmax is exactly shift-invariant.
            denom = cpool.tile([128, 1], F32, tag="denom")
            rden = cpool.tile([128, 1], F32, tag="rden")
            numer = cpool.tile([128, 1], F32, tag="numer")
            outc = cpool.tile([128, 1], F32, tag="outc")

            nc.scalar.activation(exp_s[:, H0:P], sc_t1[:, 0:H1], AF.Exp,
                                 accum_out=den2[:, 1:2])
            nc.vector.scalar_tensor_tensor(
                junk[:, H0:P], exp_s[:, H0:P], 1.0, g_t1[:, 0:H1],
                op0=ALU.mult, op1=ALU.mult, accum_out=num2[:, 1:2])
            nc.vector.tensor_reduce(numer[:], num2[:], axis=AXIS.X, op=ALU.add)
            nc.vector.tensor_reduce(denom[:], den2[:], axis=AXIS.X, op=ALU.add)
            nc.vector.reciprocal(rden[:], denom[:])
            nc.vector.tensor_mul(outc[:], numer[:], rden[:])
            nc.vector.tensor_scalar_add(outc[:], outc[:], pb_s[:])
            nc.sync.dma_start(out_d[:], outc[:])

    nc.compile()
    return nc


def make_nc(B_c=128, dve_mod=6):
    nc = bacc.Bacc("TRN2", target_bir_lowering=False, debug=False)
    build(nc, B_c=B_c, dve_mod=dve_mod)
    return nc


def perm_for(B_c=128, blocks=None):
    """perm[slot] = global b stored at SBUF slot.

    Slot k belongs to quad k//4 (the one-hot position) and column group
    k%4, so it accumulates into output partition 32*(k%4) + k//4.
    """
    k = np.arange(B_c)
    return 32 * (k % 4) + k // 4


def host_prep_consts(attn_w_w, attn_w_b, attn_h_w, attn_h_b, attn_p_w, attn_p_b):
    wT = np.ascontiguousarray(attn_w_w.T).astype(np.float16)
    bias = attn_w_b.reshape(128, 1).astype(np.float32)
    Zh = np.zeros((128, 64), np.float16)
    Zh[:, 32] = attn_h_w[0].astype(np.float16)
    Zg = np.zeros((128, 64), np.float16)
    Zg[:, 32] = attn_p_w[0].astype(np.float16)
    pb = np.full((128, 1), np.float32(attn_p_b[0]), np.float32)
    return {"wT": wT, "bias": bias, "Zh": Zh, "Zg": Zg, "pb": pb}


def host_prep_x(x_slice, blocks=None):
    # [B_c, F, E] -> two pre-shifted fp16 copies [E, B_c(perm), 60]
    xT = x_slice.transpose(2, 0, 1).astype(np.float16)
    xT = xT[:, perm_for(x_slice.shape[0]), :]
    B_c = x_slice.shape[0]
    xa = np.zeros((128, B_c, 60), np.float16)
    xa[:, :, 0:40] = xT
    xa[:, :, 40:60] = xT[:, :, 0:20]
    xb = np.zeros((128, B_c, 60), np.float16)
    xb[:, :, 0:59] = xa[:, :, 1:60]
    return np.ascontiguousarray(xa), np.ascontiguousarray(xb)


_NC_CACHE = {}


def _get_nc():
    if "nc" not in _NC_CACHE:
        _NC_CACHE["nc"] = make_nc(B_c=128)
    return _NC_CACHE["nc"]


def kernel(x, attn_w_w, attn_w_b, attn_h_w, attn_h_b, attn_p_w, attn_p_b,
           _trace=False):
    from concourse.bass_utils import run_bass_kernel_spmd
    x = np.asarray(x, np.float32)
    consts = host_prep_consts(np.asarray(attn_w_w), np.asarray(attn_w_b),
                              np.asarray(attn_h_w), np.asarray(attn_h_b),
                              np.asarray(attn_p_w), np.asarray(attn_p_b))
    in_maps = []
    for c in range(8):
        m = dict(consts)
        m["xTa"], m["xTb"] = host_prep_x(x[128 * c:128 * (c + 1)])
        in_maps.append(m)
    nc = _get_nc()
    res = run_bass_kernel_spmd(nc, in_maps, list(range(8)), trace=_trace)
    out = np.concatenate([res.results[c]["out"][:, 0] for c in range(8)])
    if _trace:
        return out.astype(np.float32), res
    return out.astype(np.float32)
